# revision 63
# baseline (speedup 1.0000x reference)
"""Block-diagonal cross-attention + MLP for trn2, 8-core data-parallel.

v8 (final): 2-block-superblock software-pipelined attention.
Graphs bin-packed in pairs into 128-row blocks (nb=13 blocks/core for
the spec data). Cross-graph + padding masking folded into the score
matmul via two extra contraction rows: score' = q.k - 25*(1 -
sigma_q*sigma_k). Per superblock (2 blocks): 4 score MMs (both
orientations) -> one exp [128,512] (ACT) -> 4 V-matmuls with mask
column -> reciprocal of rowsums + normalize broadcast-mult (DVE); per
block a PE transpose (on-chip identity, built with affine_select) into
a 2-block psum group; one eviction per group fused with the +x
residual (DVE add against the feature-major xT2 image). The loop is
staggered (scores s / vmm s-1 / transpose s-2) and the tile scheduler
reorders per-engine streams. MLP uses block-diagonal [128,128] weights
over 2-block (256-col) chunks (small groups let the MLP and output
stores start earlier, pulling in the backend-bound store drain);
bias+residual fused into the final STT
eviction; stores round-robin over sync/scalar/gpsimd queues. Inputs
are packed into few wide DMA transfers issued in consumption order:
xb chunks + the vsvt rest-chunk on sync, ONLY the first vsvt chunk on
scalar (so it completes alone instead of being interleave-dragged by
the rest, unblocking the first V-matmuls ~1us earlier), ONLY xT2 on
gpsimd, weights on sync's tail (transfer cost is dominated by
partition-line count x ~40ns at <=1KB lines, ~28GB/s per issuing
queue beyond). dtypes: fp16 images/weights, E bf16 (exp reaches
~e^46). Output: [128, nb*128] fp16 per core; host scatters per graph.
"""

from contextlib import ExitStack

import numpy as np

N_NODES = 8192
D = 64
G = 128
N_CORES = 8
BCAP = 128                  # rows per block
CD = D + 2                  # contraction rows incl. bias rows
VW = D + 1                  # v image width incl. mask column
NEG = 25.0                  # pad bias; cross-graph pairs get -2*NEG

_PROGRAM_CACHE = {}


def _build_program(nb):
    import concourse.bass as bass
    import concourse.tile as tile
    from concourse import bacc, mybir

    fp32 = mybir.dt.float32
    fp16 = mybir.dt.float16
    bf16 = mybir.dt.bfloat16
    rows = nb * BCAP
    nc = bacc.Bacc("TRN2", target_bir_lowering=False, debug=False)

    xb = nc.declare_dram_parameter("xb", [CD, 2 * rows], fp16, isOutput=False)
    vsvt = nc.declare_dram_parameter("vsvt", [BCAP, 2 * nb * VW], bf16,
                                     isOutput=False)
    xT2 = nc.declare_dram_parameter("xT2", [2 * D, rows], fp16,
                                    isOutput=False)
    wpack = nc.declare_dram_parameter("wpack", [2 * D, 4 * D], fp16,
                                      isOutput=False)
    bpack = nc.declare_dram_parameter("bpack", [2 * D, 2], fp32, isOutput=False)
    outp = nc.declare_dram_parameter("outp", [2 * D, rows], fp16, isOutput=True)

    AF = mybir.ActivationFunctionType
    ALU = mybir.AluOpType

    with tile.TileContext(nc) as tc, ExitStack() as ctx:
        singles = ctx.enter_context(tc.tile_pool(name="singles", bufs=1))
        epool = ctx.enter_context(tc.tile_pool(name="epool", bufs=4))
        rpool = ctx.enter_context(tc.tile_pool(name="rpool", bufs=4))

        sb_xb = singles.tile([CD, 2 * rows], fp16, tag="xb")
        sb_vv = singles.tile([BCAP, 2 * nb * VW], bf16, tag="vsvt")
        sb_x2 = singles.tile([2 * D, rows], fp16, tag="xT2")
        sb_id = singles.tile([BCAP, BCAP], fp16, tag="ident")
        sb_w = singles.tile([2 * D, 4 * D], fp16, tag="wpack")
        sb_b = singles.tile([2 * D, 2], fp32, tag="bpack")
        sb_eT = singles.tile([2 * D, rows], fp16, tag="eT")
        sb_h = singles.tile([2 * D, rows], fp16, tag="h")
        sb_out = singles.tile([2 * D, rows], fp16, tag="out")

        def xs_blk(b):
            return sb_xb[:, 2 * b * BCAP:2 * b * BCAP + BCAP]

        def xt_blk(b):
            return sb_xb[:, 2 * b * BCAP + BCAP:2 * (b + 1) * BCAP]

        # --- input DMA first on every queue, in consumption order ---
        # sync: xb in 4 chunks sized to the superblock consumption order
        # scalar: vsvt in 2 chunks
        # gpsimd: xT2 in 2 chunks + weights/biases + on-chip identity
        xcuts = [0, 4 * BCAP, 8 * BCAP, 16 * BCAP, 2 * rows]
        xcuts = sorted(set(min(c, 2 * rows) for c in xcuts))
        for a, b in zip(xcuts[:-1], xcuts[1:]):
            nc.sync.dma_start(out=sb_xb[:, a:b], in_=xb[:, a:b])
        vp = min(8 * VW, 2 * nb * VW)   # vsvt cols for blocks 0-3
        # scalar carries ONLY the first vsvt chunk so it completes alone
        # (~12us instead of being interleave-dragged to the end of all
        # 300KB); the rest rides sync's idle tail after the xb chunks
        nc.scalar.dma_start(out=sb_vv[:, 0:vp], in_=vsvt[:, 0:vp])
        if vp < 2 * nb * VW:
            nc.sync.dma_start(out=sb_vv[:, vp:], in_=vsvt[:, vp:])
        xp = min(4 * BCAP, rows)        # xT2 cols for blocks 0-3
        # gpsimd carries ONLY xT2 so its first chunk isn't dragged by the
        # weights' bytes; weights ride sync's tail (land well before the
        # MLP needs them) - same un-drag mechanism as the vsvt fix above
        nc.gpsimd.dma_start(out=sb_x2[:, 0:xp], in_=xT2[:, 0:xp])
        if xp < rows:
            nc.gpsimd.dma_start(out=sb_x2[:, xp:], in_=xT2[:, xp:])
        nc.sync.dma_start(out=sb_w, in_=wpack[:, :])
        nc.sync.dma_start(out=sb_b, in_=bpack[:, :])
        # identity for PE transposes, built on-chip
        nc.gpsimd.memset(sb_id, 1.0)
        nc.gpsimd.affine_select(
            out=sb_id, in_=sb_id, pattern=[[-1, BCAP]],
            compare_op=mybir.AluOpType.is_equal, fill=0.0,
            base=0, channel_multiplier=1)

        GRP = 2                # blocks per transpose group / MLP chunk
        n3 = (nb + GRP - 1) // GRP

        with tc.tile_pool(name="ps_sc", bufs=4, space="PSUM") as ps_sc, \
             tc.tile_pool(name="ps_o", bufs=2, space="PSUM") as ps_o, \
             tc.tile_pool(name="ps_t", bufs=2, space="PSUM") as ps_t:
            ps_m = ps_sc       # MLP psum tiles share the score ring

            ns2 = (nb + 1) // 2    # 2-block superblocks
            sc_t = [None] * ns2
            et_t = [None] * ns2
            o_t = [None] * ns2
            er_t = [None] * ns2
            tp_t = [None] * n3

            def sblocks(s):
                return list(range(2 * s, min(2 * s + 2, nb)))

            for i in range(ns2 + 2):
                s0 = i          # scores + exp (superblock)
                s1 = i - 1      # vmm + recip + mult (superblock)
                s2 = i - 2      # per-block transposes + group evicts

                if s0 < ns2:
                    g = len(sblocks(s0))
                    sc = ps_sc.tile([BCAP, 4 * BCAP], fp32, tag="sc",
                                    name="sc")
                    sc_t[s0] = sc
                    for j, b in enumerate(sblocks(s0)):
                        q = j * 2 * BCAP
                        nc.tensor.matmul(sc[:, q:q + BCAP], xs_blk(b),
                                         xt_blk(b), start=True, stop=True)
                        nc.tensor.matmul(sc[:, q + BCAP:q + 2 * BCAP],
                                         xt_blk(b), xs_blk(b),
                                         start=True, stop=True)
                if 0 <= s1 < ns2:
                    g = len(sblocks(s1))
                    et = et_t[s1]
                    o = ps_o.tile([BCAP, 4 * VW], fp32, tag="o", name="o")
                    o_t[s1] = o
                    for j, b in enumerate(sblocks(s1)):
                        va_s = 2 * b * VW
                        va_t = va_s + VW
                        q = j * 2 * BCAP
                        v = j * 2 * VW
                        # src out (queries=xs): E_st rows are xt -> lhsT
                        nc.tensor.matmul(o[:, v:v + VW],
                                         et[:, q + BCAP:q + 2 * BCAP],
                                         sb_vv[:, va_t:va_t + VW],
                                         start=True, stop=True)
                        # tar out (queries=xt)
                        nc.tensor.matmul(o[:, v + VW:v + 2 * VW],
                                         et[:, q:q + BCAP],
                                         sb_vv[:, va_s:va_s + VW],
                                         start=True, stop=True)
                if 0 <= s2 < ns2:
                    for j, b in enumerate(sblocks(s2)):
                        c3, j3 = divmod(b, GRP)
                        if j3 == 0:
                            tp_t[c3] = ps_t.tile([BCAP, GRP * BCAP], fp16,
                                                 tag="tp", name="tp")
                        nc.tensor.transpose(
                            tp_t[c3][:, j3 * BCAP:(j3 + 1) * BCAP],
                            er_t[s2][:, j * BCAP:(j + 1) * BCAP], sb_id)

                if s0 < ns2:
                    g = len(sblocks(s0))
                    et = epool.tile([BCAP, 4 * BCAP], bf16, tag="E", name="E")
                    et_t[s0] = et
                    nc.scalar.activation(out=et[:, 0:g * 2 * BCAP],
                                         in_=sc_t[s0][:, 0:g * 2 * BCAP],
                                         func=AF.Exp)

                if 0 <= s1 < ns2:
                    g = len(sblocks(s1))
                    o3 = o_t[s1][:, 0:g * 2 * VW].rearrange(
                        "p (s v) -> p s v", v=VW)
                    rc = rpool.tile([BCAP, 4], fp32, tag="rc", name="rc")
                    nc.vector.reciprocal(
                        out=rc[:, 0:2 * g].rearrange("p (s v) -> p s v", v=1),
                        in_=o3[:, :, D:D + 1])
                    er = rpool.tile([BCAP, 4 * D], fp16, tag="er", name="er")
                    er_t[s1] = er
                    nc.vector.tensor_tensor(
                        out=er[:, 0:g * 2 * D].rearrange(
                            "p (s v) -> p s v", v=D),
                        in0=o3[:, :, 0:D],
                        in1=rc[:, 0:2 * g].rearrange("p (s v) -> p s v", v=1)
                            .broadcast_to([BCAP, 2 * g, D]),
                        op=ALU.mult)
                if 0 <= s2 < ns2:
                    for b in sblocks(s2):
                        if b % GRP == GRP - 1 or b == nb - 1:
                            # one eviction + residual add per group
                            c3 = b // GRP
                            c = c3 * GRP * BCAP
                            w3 = min(GRP * BCAP, rows - c)
                            nc.vector.tensor_tensor(
                                out=sb_eT[:, c:c + w3], in0=tp_t[c3][:, 0:w3],
                                in1=sb_x2[:, c:c + w3], op=ALU.add)

            # ---- packed MLP over [128, rows], GRP-block chunks ----
            mlp_ch = GRP * BCAP
            sb_w1 = sb_w[:, 0:2 * D]
            sb_w2 = sb_w[:, 2 * D:4 * D]
            sb_b1 = sb_b[:, 0:1]
            sb_b2 = sb_b[:, 1:2]
            for c in range(0, rows, mlp_ch):
                w = min(mlp_ch, rows - c)
                hp = ps_m.tile([2 * D, 4 * BCAP], fp32, tag="sc",
                               name="hp")
                nc.tensor.matmul(hp[:, 0:w], sb_w1, sb_eT[:, c:c + w],
                                 start=True, stop=True)
                nc.scalar.activation(out=sb_h[:, c:c + w], in_=hp[:, 0:w],
                                     func=AF.Relu, bias=sb_b1, scale=1.0)
            st_eng = [nc.sync, nc.scalar, nc.gpsimd]
            for k, c in enumerate(range(0, rows, mlp_ch)):
                w = min(mlp_ch, rows - c)
                op2 = ps_m.tile([2 * D, 4 * BCAP], fp32, tag="sc",
                                name="op2")
                nc.tensor.matmul(op2[:, 0:w], sb_w2, sb_h[:, c:c + w],
                                 start=True, stop=True)
                nc.vector.scalar_tensor_tensor(
                    out=sb_out[:, c:c + w], in0=op2[:, 0:w], scalar=sb_b2,
                    in1=sb_eT[:, c:c + w], op0=ALU.add, op1=ALU.add)
                st_eng[k % 3].dma_start(out=outp[:, c:c + w],
                                        in_=sb_out[:, c:c + w])

    nc.compile()
    return nc


def _pack_blocks(cnt_s, cnt_t):
    """Pair graphs into 128-row blocks. Returns list of blocks, each a
    list of (graph_id, row_offset)."""
    n = np.maximum(cnt_s, cnt_t)
    order = np.argsort(n, kind="stable")
    lo, hi = 0, len(order) - 1
    blocks = []
    while lo <= hi:
        g_hi = order[hi]
        if lo < hi and n[order[lo]] + n[g_hi] <= BCAP:
            g_lo = order[lo]
            blocks.append([(int(g_hi), 0), (int(g_lo), int(n[g_hi]))])
            lo += 1
        else:
            blocks.append([(int(g_hi), 0)])
        hi -= 1
    return blocks


def _shard_inputs(x_src, batch_src, x_tar, batch_tar, w1, b1, w2, b2):
    bs = np.asarray(batch_src).astype(np.int64)
    bt = np.asarray(batch_tar).astype(np.int64)
    xs = np.asarray(x_src, dtype=np.float32)
    xt = np.asarray(x_tar, dtype=np.float32)

    bnd_s = np.searchsorted(bs, np.arange(G + 1))
    bnd_t = np.searchsorted(bt, np.arange(G + 1))
    cnt_s = np.diff(bnd_s)
    cnt_t = np.diff(bnd_t)
    if np.maximum(cnt_s, cnt_t).max(initial=0) > BCAP:
        return None, None, (bnd_s, bnd_t, cnt_s, cnt_t)

    blocks = _pack_blocks(cnt_s, cnt_t)
    nb = (len(blocks) + N_CORES - 1) // N_CORES
    rows = nb * BCAP
    core_blocks = [[] for _ in range(N_CORES)]
    for i, blk in enumerate(blocks):
        core_blocks[i % N_CORES].append(blk)

    w1a = np.asarray(w1, dtype=np.float32)
    w2a = np.asarray(w2, dtype=np.float32)
    b1a = np.asarray(b1, dtype=np.float32).reshape(D)
    b2a = np.asarray(b2, dtype=np.float32).reshape(D)
    wpack = np.zeros((2 * D, 4 * D), dtype=np.float16)
    wpack[:D, :D] = w1a; wpack[D:, D:2 * D] = w1a
    wpack[:D, 2 * D:3 * D] = w2a; wpack[D:, 3 * D:] = w2a
    bpack = np.stack([np.concatenate([b1a, b1a]),
                      np.concatenate([b2a, b2a])], axis=1).astype(np.float32)

    to_bf16 = _bf16_caster()

    in_maps = []
    placement = []   # per core: list of (g, row_off_in_core_img)
    for c in range(N_CORES):
        blks = core_blocks[c]
        xs_img = np.zeros((CD, rows), dtype=np.float16)
        xt_img = np.zeros((CD, rows), dtype=np.float16)
        vs_img = np.zeros((BCAP, nb * VW), dtype=np.float32)
        vt_img = np.zeros((BCAP, nb * VW), dtype=np.float32)
        vs_img[:, VW - 1::VW] = 1.0   # mask column: ones everywhere
        vt_img[:, VW - 1::VW] = 1.0
        # ones bias row on ALL query rows (incl. padding): pad queries then
        # score -NEG against every key, so their exp(~0) rows don't pollute
        # the transposed-side rowsums through the all-ones mask column
        xs_img[D, :] = 1.0
        place = []
        for bi, blk in enumerate(blks):
            col = bi * BCAP
            for gi, (g, off) in enumerate(blk):
                ns, nt = cnt_s[g], cnt_t[g]
                sig = 1.0 if gi == 0 else -1.0
                xs_img[:D, col + off:col + off + ns] = xs[bnd_s[g]:bnd_s[g + 1]].T
                xt_img[:D, col + off:col + off + nt] = xt[bnd_t[g]:bnd_t[g + 1]].T
                # bias rows: score' = q.k + 1_q*(-NEG)_k + sig_q*(NEG*sig)_k
                xs_img[D + 1, col + off:col + off + ns] = sig
                xt_img[D, col + off:col + off + nt] = -NEG
                xt_img[D + 1, col + off:col + off + nt] = NEG * sig
                vs_img[off:off + ns, bi * VW:bi * VW + D] = xs[bnd_s[g]:bnd_s[g + 1]]
                vt_img[off:off + nt, bi * VW:bi * VW + D] = xt[bnd_t[g]:bnd_t[g + 1]]
                place.append((int(g), col + off))
        for bi in range(len(blks)):
            col = bi * BCAP
            m = xt_img[D, col:col + BCAP] == 0.0
            xt_img[D, col:col + BCAP][m] = -NEG
        # interleave xs/xt blocks: [xs_b0 | xt_b0 | xs_b1 | xt_b1 | ...]
        xb_img = np.empty((CD, nb, 2, BCAP), dtype=np.float16)
        xb_img[:, :, 0, :] = xs_img.reshape(CD, nb, BCAP)
        xb_img[:, :, 1, :] = xt_img.reshape(CD, nb, BCAP)
        xb_img = xb_img.reshape(CD, 2 * rows)
        vsvt_img = np.empty((BCAP, nb, 2, VW), dtype=np.float32)
        vsvt_img[:, :, 0, :] = vs_img.reshape(BCAP, nb, VW)
        vsvt_img[:, :, 1, :] = vt_img.reshape(BCAP, nb, VW)
        vsvt_img = vsvt_img.reshape(BCAP, 2 * nb * VW)
        # feature-major stacked residual image
        xT2_img = np.concatenate([xs_img[:D], xt_img[:D]], axis=0)
        in_maps.append({
            "xb": xb_img,
            "vsvt": to_bf16(vsvt_img),
            "xT2": xT2_img,
            "wpack": wpack, "bpack": bpack,
        })
        placement.append(place)
    meta = (bnd_s, bnd_t, cnt_s, cnt_t, placement, nb)
    return in_maps, nb, meta


def _bf16_caster():
    import ml_dtypes
    return lambda a: a.astype(ml_dtypes.bfloat16)


def _numpy_fallback(x_src, batch_src, x_tar, batch_tar, w1, b1, w2, b2):
    bs = np.asarray(batch_src); bt = np.asarray(batch_tar)
    xs = np.asarray(x_src, dtype=np.float64); xt = np.asarray(x_tar, dtype=np.float64)
    mask = bs[:, None] == bt[None, :]

    def attend(q, kv, m):
        s = np.where(m, q @ kv.T, -1.0e9)
        s = s - s.max(axis=1, keepdims=True)
        e = np.exp(s)
        a = e / e.sum(axis=1, keepdims=True)
        out = a @ kv + q
        return np.where(m.any(axis=1, keepdims=True), out, 0.0)

    def mlp(x):
        return np.maximum(x @ w1 + b1, 0.0) @ w2 + b2 + x

    es = mlp(attend(xs, xt, mask))
    et = mlp(attend(xt, xs, mask.T))
    return et.astype(np.float32), es.astype(np.float32)


def kernel(x_src, batch_src, x_tar, batch_tar, w1, b1, w2, b2):
    in_maps, nb, meta = _shard_inputs(
        x_src, batch_src, x_tar, batch_tar, w1, b1, w2, b2)
    if in_maps is None:  # a graph overflowed BCAP; never happens for spec data
        return _numpy_fallback(
            x_src, batch_src, x_tar, batch_tar, w1, b1, w2, b2)
    bnd_s, bnd_t, cnt_s, cnt_t, placement, nb = meta

    import os
    from concourse import bass_utils
    key = nb
    if key not in _PROGRAM_CACHE:
        _PROGRAM_CACHE[key] = _build_program(nb)
    nc = _PROGRAM_CACHE[key]
    trace = bool(os.environ.get("KERNEL_TRACE"))
    res = bass_utils.run_bass_kernel_spmd(
        nc, in_maps, core_ids=list(range(N_CORES)), trace=trace)
    _PROGRAM_CACHE["last_result"] = res

    # rows whose graph has no counterpart: reference yields mlp(0)
    w1a = np.asarray(w1, np.float32); b1a = np.asarray(b1, np.float32)
    w2a = np.asarray(w2, np.float32); b2a = np.asarray(b2, np.float32)
    mlp0 = np.maximum(b1a, 0.0) @ w2a + b2a

    embed_src = np.zeros((N_NODES, D), dtype=np.float32)
    embed_tar = np.zeros((N_NODES, D), dtype=np.float32)
    for c in range(N_CORES):
        op = np.asarray(res.results[c]["outp"]).astype(np.float32)
        for g, off in placement[c]:
            ns, nt = cnt_s[g], cnt_t[g]
            if ns > 0:
                embed_src[bnd_s[g]:bnd_s[g] + ns] = (
                    op[0:D, off:off + ns].T if nt > 0 else mlp0)
            if nt > 0:
                embed_tar[bnd_t[g]:bnd_t[g] + nt] = (
                    op[D:2 * D, off:off + nt].T if ns > 0 else mlp0)
    return embed_tar, embed_src
